# revision 3
# baseline (speedup 1.0000x reference)
"""Multi-head attention (B=8, T=1024, D=768, H=12) on 8 TRN2 NeuronCores.

Sharding: data-parallel over batch - one batch element per core, no
collectives.

v3: flat software-pipelined loop over 48 global (pair, tk) steps, with
all PE work packed via PE-array tiling (measured ~2.1x for pairs, ~4.5x
for the 4-way denominators):
  scores  : both heads row-paired (contraction 64 -> row groups 0:63 /
            64:127), 4 matmuls into one 4-bank [128,2048] PSUM tile
  exp     : ONE FD=2048 ACTIVATE per step (amortizes ACT overhead)
  attnV   : both heads col-paired into one PSUM tile (out partitions
            0:64 / 64:128)
  denoms  : ones[128,32] lhsT, 4-way col-tiled into one bank; every
            partition written so one DVE reciprocal covers all four
  fillers : V-proj halves, QK-proj halves (bias on DVE), and output-proj
            partial-contraction groups scheduled into specific steps;
            scores are emitted first in each step so exp never waits
  normalize: recip -> 3 row-extract DMAs -> 4 gpsimd broadcasts -> 2 DVE
            multiplies; attnV lags scores by a variable 4..6 steps so the
            chain never stalls the next pair's attnV (oacc/den WAR)
  tail    : yacc (ko 0..3, bf16) folded into PSUM via an identity matmul,
            ko4 prefilled during the last normalize chain, bf16 writeout

PSUM (8 banks): sc [128,2048] (4) + oacc [128,1024] (2) +
den [128,512] (1) + stg [128,512] (1).
"""

import numpy as np
import ml_dtypes

import concourse.bass as bass
import concourse.mybir as mybir
import concourse.tile as tile
from concourse import bacc
from concourse import bass_utils

BF16 = mybir.dt.bfloat16
F32 = mybir.dt.float32

B, T, D = 8, 1024, 768
H, HD = 12, 64
P = 128
ND = D // P           # 6 d-tiles
NT = T // P           # 8 t-tiles
NPAIR = H // 2        # 6 head pairs
NSTEP = NPAIR * NT    # 48 global steps
SCALE = HD ** -0.5
LAG = 5               # attnV lags scores by LAG global steps


def build():
    nc = bacc.Bacc("TRN2", target_bir_lowering=False, debug=False, num_devices=8)

    xT_d = nc.dram_tensor("xT", [D, T], BF16, kind="ExternalInput").ap()
    wqk_d = nc.dram_tensor("wqk", [D, 2 * D], BF16, kind="ExternalInput").ap()
    wv_d = nc.dram_tensor("wv", [D, D], BF16, kind="ExternalInput").ap()
    projT_d = nc.dram_tensor("projT", [D, D], BF16, kind="ExternalInput").ap()
    qkb_d = nc.dram_tensor("qkb", [P, 2 * ND], F32, kind="ExternalInput").ap()
    pb2_d = nc.dram_tensor("pb2", [P, ND], F32, kind="ExternalInput").ap()
    ident_d = nc.dram_tensor("ident", [P, P], BF16, kind="ExternalInput").ap()
    yT_d = nc.dram_tensor("yT", [D, T], BF16, kind="ExternalOutput").ap()

    with tile.TileContext(nc) as tc:
        with tc.tile_pool(name="const", bufs=1) as const, \
             tc.tile_pool(name="work", bufs=4) as work, \
             tc.tile_pool(name="psc", bufs=2, space="PSUM") as psc, \
             tc.tile_pool(name="pso", bufs=1, space="PSUM") as pso:

            # ---- resident SBUF tensors ----
            xT_sb = const.tile([P, ND, T], BF16)
            wv_sb = const.tile([P, ND, D], BF16)
            wqk_sb = const.tile([P, ND, 2 * D], BF16)
            projT_sb = const.tile([P, ND, D], BF16)
            qkb_sb = const.tile([P, 2 * ND], F32)
            pb2_sb = const.tile([P, ND], F32)
            QKT_sb = const.tile([P, 2 * ND, T], BF16)
            V_sb = const.tile([P, NT, D], BF16)
            aoT_sb = const.tile([P, ND, T], BF16)
            yacc_sb = const.tile([P, 2 * ND, 512], BF16)
            ones_sb = const.tile([P, 32], BF16)
            ident_sb = const.tile([P, P], BF16)

            nc.vector.memset(ones_sb[:], 1.0)
            nc.sync.dma_start(ident_sb[:], ident_d)

            # DMA priority: xT then the first Q/K weight tiles (unblocks the
            # first scores), then wv, then pair-1's Q/K tiles
            xT_r = xT_d.rearrange("(ko p) t -> p ko t", p=P)
            wv_r = wv_d.rearrange("(ko p) j -> p ko j", p=P)
            wqk_r = wqk_d.rearrange("(ko p) j -> p ko j", p=P)
            for c in range(3):
                nc.sync.dma_start(xT_sb[:, 2 * c:2 * c + 2, :],
                                  xT_r[:, 2 * c:2 * c + 2, :])
            for jt in (0, ND):
                nc.sync.dma_start(
                    wqk_sb[:, :, jt * P:(jt + 1) * P], wqk_r[:, :, jt * P:(jt + 1) * P])
            nc.sync.dma_start(qkb_sb[:], qkb_d)
            for c in range(3):
                nc.sync.dma_start(wv_sb[:, 2 * c:2 * c + 2, :],
                                  wv_r[:, 2 * c:2 * c + 2, :])
            for jt in (1, ND + 1):
                nc.sync.dma_start(
                    wqk_sb[:, :, jt * P:(jt + 1) * P], wqk_r[:, :, jt * P:(jt + 1) * P])

            # warm the exp table set immediately (no DMA dependency)
            warm = work.tile([1, 12], F32, tag="warm", bufs=1)
            nc.scalar.activation(warm[:], ones_sb[0:1, 0:12], mybir.ActivationFunctionType.Exp)


            def emit_v_half(t, jc):
                j0, jn = ((0, 512), (512, 256))[jc]
                ps = pso.tile([P, 512], F32, tag="stg", name=f"psvh_{t}_{jc}")
                for d in range(ND):
                    nc.tensor.matmul(
                        ps[:, :jn],
                        xT_sb[:, d, t * P:(t + 1) * P],
                        wv_sb[:, d, j0:j0 + jn],
                        start=(d == 0), stop=(d == ND - 1),
                    )
                nc.vector.tensor_copy(out=V_sb[:, t, j0:j0 + jn], in_=ps[:, :jn])

            def emit_qk_full(jt):
                ps = psc.tile([P, T], F32, tag="sc", name=f"psqk_{jt}", bufs=1)
                mm = None
                for d in range(ND):
                    for tq in range(2):
                        mm = nc.tensor.matmul(
                            ps[:, tq * 512:(tq + 1) * 512],
                            wqk_sb[:, d, jt * P:(jt + 1) * P],
                            xT_sb[:, d, tq * 512:(tq + 1) * 512],
                            start=(d == 0), stop=(d == ND - 1),
                        )
                nc.vector.tensor_scalar_add(
                    QKT_sb[:, jt, :], ps[:], qkb_sb[:, jt:jt + 1])
                return mm

            def emit_qk_half(jt, tq):
                ps = pso.tile([P, 512], F32, tag="stg", name=f"psqkh_{jt}_{tq}")
                for d in range(ND):
                    nc.tensor.matmul(
                        ps[:],
                        wqk_sb[:, d, jt * P:(jt + 1) * P],
                        xT_sb[:, d, tq * 512:(tq + 1) * 512],
                        start=(d == 0), stop=(d == ND - 1),
                    )
                nc.vector.tensor_scalar_add(
                    QKT_sb[:, jt, tq * 512:(tq + 1) * 512], ps[:],
                    qkb_sb[:, jt:jt + 1])

            def emit_proj(dt, tq, kos, first, last):
                # partial output-projection: accumulate contraction tiles kos
                # into yacc (bias folded on the first pass, in-place adds after)
                ps = pso.tile([P, 512], F32, tag="stg", name=f"pj_{dt}_{tq}_{kos[0]}")
                for n, ko in enumerate(kos):
                    nc.tensor.matmul(
                        ps[:],
                        projT_sb[:, ko, dt * P:(dt + 1) * P],
                        aoT_sb[:, ko, tq * 512:(tq + 1) * 512],
                        start=(n == 0), stop=(n == len(kos) - 1),
                    )
                ya = yacc_sb[:, 2 * dt + tq, :]
                if first:
                    nc.vector.tensor_scalar_add(ya, ps[:], pb2_sb[:, dt:dt + 1])
                else:
                    nc.vector.tensor_tensor(ya, ps[:], ya, mybir.AluOpType.add)

            # ---- upfront: pair-0 Q/K tiles (rides the input DMA) ----
            anchor = emit_qk_full(0)
            emit_qk_full(ND)

            # bulk weight DMAs gated behind the first QK tile (software-DGE)
            from concourse.tile_rust import add_dep_helper
            bulk = [
                nc.gpsimd.dma_start(
                    wqk_sb[:, :, 2 * P:ND * P], wqk_r[:, :, 2 * P:ND * P]),
                nc.gpsimd.dma_start(
                    wqk_sb[:, :, (ND + 2) * P:], wqk_r[:, :, (ND + 2) * P:]),
                nc.gpsimd.dma_start(
                    projT_sb[:], projT_d.rearrange("(ko p) j -> p ko j", p=P)),
                nc.gpsimd.dma_start(pb2_sb[:], pb2_d),
            ]
            for b in bulk:
                add_dep_helper(b.ins, anchor.ins, sync=True,
                               reason="bulk weight DMA after first QK tile")

            # ---- filler schedule: gk -> list of (fn, args) ----
            fillers = {}

            def sched(gk, fn, *args):
                fillers.setdefault(gk, []).append((fn, args))

            for t in range(4):                       # V t=0..3 early
                sched(t, emit_v_half, t, 0)
                sched(t, emit_v_half, t, 1)
            sched(4, emit_qk_half, 1, 0)             # pair-1 Q tile
            sched(4, emit_qk_half, 1, 1)
            sched(5, emit_qk_half, ND + 1, 0)        # pair-1 K tile
            sched(5, emit_qk_half, ND + 1, 1)
            for t in range(4, NT):                   # V t=4..7
                sched(2 + t, emit_v_half, t, 0)
                sched(2 + t, emit_v_half, t, 1)
            # normalize lands at gk 12,20,28,36,44 - keep those steps
            # filler-free so the stg ring never waits behind the boundary
            # DVE burst
            qk_slots = [10, 11, 13, 14, 15, 16, 17, 18, 19, 21, 22, 23,
                        24, 25, 26, 27]
            qk_jobs = [(jt, tq) for jt in (2, ND + 2, 3, ND + 3, 4, ND + 4,
                                           5, ND + 5) for tq in range(2)]
            for slot, (jt, tq) in zip(qk_slots, qk_jobs):
                sched(slot, emit_qk_half, jt, tq)
            # output projection partials: ko 0,1 after aoT(1) (~gk21);
            # ko 2,3 after aoT(3) (~gk37); ko 4,5 + writeout in the tail
            pj = [(dt, tq) for dt in range(ND) for tq in range(2)]
            p1_slots = [29, 29, 30, 30, 31, 31, 32, 32, 33, 34, 35, 37]
            p2_slots = [39, 39, 40, 40, 41, 41, 42, 42, 43, 43, 45, 45]
            for n, (dt, tq) in enumerate(pj):
                sched(p1_slots[n], emit_proj, dt, tq, (0, 1), True, False)
                sched(p2_slots[n], emit_proj, dt, tq, (2, 3), False, False)

            # ---- attention pipeline over 48 global steps ----
            pair_state = {}

            def emit_scores(gk):
                i, tk = divmod(gk, NT)
                if tk == 0:
                    oacc = pso.tile([P, T], F32, tag="oacc", name=f"oacc_{i}")
                    den = pso.tile([P, 512], F32, tag="den", name=f"den_{i}")
                    pair_state[i] = (oacc, den, {})
                at_tiles = pair_state[i][2]
                # one 4-bank score tile holds both heads -> a single FD=2048
                # exp per step (amortizes the ACT per-instruction overhead)
                sc = psc.tile([P, 2 * T], F32, tag="sc", name=f"sc_{i}_{tk}",
                              bufs=1)
                for tq in range(2):
                    for hh in range(2):
                        p0 = 64 * hh
                        nc.tensor.matmul(
                            sc[:, hh * T + tq * 512:hh * T + (tq + 1) * 512],
                            QKT_sb[p0:p0 + 64, ND + i, tk * P:(tk + 1) * P],
                            QKT_sb[p0:p0 + 64, i, tq * 512:(tq + 1) * 512],
                        )
                at = work.tile([P, 2 * T], BF16, tag="at",
                               name=f"at_{i}_{tk}", bufs=8)
                nc.scalar.activation(
                    at[:], sc[:], mybir.ActivationFunctionType.Exp,
                    scale=SCALE)
                at_tiles[tk] = at

            def emit_attnv(gk):
                i, tk = divmod(gk, NT)
                oacc, den, at_tiles = pair_state[i]
                at = at_tiles.pop(tk)
                for tq in range(2):
                    for hh in range(2):
                        h = 2 * i + hh
                        nc.tensor.matmul(
                            oacc[64 * hh:64 * hh + 64, tq * 512:(tq + 1) * 512],
                            V_sb[:, tk, h * HD:(h + 1) * HD],
                            at[:, hh * T + tq * 512:hh * T + (tq + 1) * 512],
                            start=(tk == 0), stop=(tk == NT - 1),
                        )
                for tq in range(2):
                    for hh in range(2):
                        g = hh + 2 * tq
                        nc.tensor.matmul(
                            den[32 * g:32 * g + 32, 0:512],
                            ones_sb[:, 0:32],
                            at[:, hh * T + tq * 512:hh * T + (tq + 1) * 512],
                            start=(tk == 0), stop=(tk == NT - 1),
                            tile_position=(0, 32 * g),
                            skip_group_check=True,
                        )

            def emit_normalize_a(i):
                # recip + row extraction + broadcasts (DVE cost: recip only)
                oacc, den, _ = pair_state[i]
                rsb = work.tile([P, 512], F32, tag="rsb", name=f"rsb_{i}", bufs=2)
                nc.vector.reciprocal_approx_fast(rsb[:], den[:])
                rext = work.tile([P, 3, 512], F32, tag="rext",
                                 name=f"rext_{i}", bufs=2)
                nc.sync.dma_start(rext[0:1, 0, :], rsb[32:33, :])
                nc.sync.dma_start(rext[0:1, 1, :], rsb[64:65, :])
                nc.sync.dma_start(rext[0:1, 2, :], rsb[96:97, :])
                rbc0 = work.tile([64, T], F32, tag="rbc0", name=f"rbc0_{i}", bufs=2)
                rbc1 = work.tile([64, T], F32, tag="rbc1", name=f"rbc1_{i}", bufs=2)
                nc.gpsimd.partition_broadcast(rbc0[:, 0:512], rsb[0:1, :])
                nc.gpsimd.partition_broadcast(rbc0[:, 512:1024], rext[0:1, 1, :])
                nc.gpsimd.partition_broadcast(rbc1[:, 0:512], rext[0:1, 0, :])
                nc.gpsimd.partition_broadcast(rbc1[:, 512:1024], rext[0:1, 2, :])
                pair_state[i] = (oacc, den, (rbc0, rbc1))

            def emit_normalize_b(i):
                # emitted ~2 steps after part a so independent filler DVE ops
                # run ahead of the broadcast-gated multiplies in the DVE FIFO
                oacc, den, (rbc0, rbc1) = pair_state[i]
                nc.vector.tensor_tensor(
                    aoT_sb[0:64, i, :], oacc[0:64, :], rbc0[:],
                    mybir.AluOpType.mult)
                nc.vector.tensor_tensor(
                    aoT_sb[64:128, i, :], oacc[64:128, :], rbc1[:],
                    mybir.AluOpType.mult)

            # variable attnV lag: finish a pair's attnV early (lag 4) and
            # start the next pair's late (lag 6) so the normalize chain gets
            # a ~3-step window before the next pair needs the oacc/den banks
            def lag_of(j):
                tk = j % NT
                return 4 if tk >= NT - 2 else (6 if tk <= 1 else 5)

            attnv_at = {}
            for j in range(NSTEP):
                attnv_at.setdefault(j + lag_of(j), []).append(j)

            for gk in range(NSTEP + 7):
                if gk < NSTEP:
                    emit_scores(gk)
                for fn, args in fillers.get(gk, []):
                    fn(*args)
                for j in attnv_at.get(gk, []):
                    emit_attnv(j)
                    if j % NT == NT - 1:
                        emit_normalize_a(j // NT)
                for j in attnv_at.get(gk - 2, []):
                    if j % NT == NT - 1:
                        emit_normalize_b(j // NT)

            # ---- tail: ko 4+5 as one PSUM group per output half, single
            # DVE add each. Rotate over 3 PSUM regions (stg + 2 freed sc
            # slots) with the ko4 matmuls software-pipelined 3 ahead so they
            # fill the last normalize-chain window (ko5 waits on aoT[5]).
            tail_ps = []

            def tail_bank(n):
                if n % 2 == 0:
                    return pso.tile([P, 512], F32, tag="stg", name=f"pt_{n}")
                return psc.tile([P, T], F32, tag="sc", name=f"pt_{n}",
                                bufs=1)[:, 0:512]

            def tail_ko4(n):
                dt, tq = pj[n]
                ps = tail_bank(n)
                tail_ps.append(ps)
                nc.tensor.matmul(
                    ps, projT_sb[:, 4, dt * P:(dt + 1) * P],
                    aoT_sb[:, 4, tq * 512:(tq + 1) * 512],
                    start=True, stop=False)
                # fold the ko0..3 partial in via an identity matmul so the
                # writeout needs only a (2x-mode) bf16 copy, no DVE add
                nc.tensor.matmul(
                    ps, ident_sb[:],
                    yacc_sb[:, 2 * dt + tq, :],
                    start=False, stop=False)

            def tail_ko5(n):
                dt, tq = pj[n]
                ps = tail_ps[n]
                nc.tensor.matmul(
                    ps, projT_sb[:, 5, dt * P:(dt + 1) * P],
                    aoT_sb[:, 5, tq * 512:(tq + 1) * 512],
                    start=False, stop=True)
                yt = work.tile([P, 512], BF16, tag="yt", name=f"ytt_{n}", bufs=4)
                nc.vector.tensor_copy(out=yt[:], in_=ps)
                nc.sync.dma_start(
                    yT_d[dt * P:(dt + 1) * P, tq * 512:(tq + 1) * 512], yt[:])

            tail_ko4(0)
            tail_ko4(1)
            tail_ko4(2)
            for n in range(12):
                tail_ko5(n)
                if n + 3 < 12:
                    tail_ko4(n + 3)

    nc.compile()
    return nc


def prep_inputs(x, qkv_w, qkv_b, proj_w, proj_b):
    """Host-side layout prep. Returns per-core input maps."""
    bf = ml_dtypes.bfloat16
    wqkvT = np.ascontiguousarray(qkv_w.T)          # [768, 2304] f32
    wqk = wqkvT[:, :2 * D].astype(bf)
    wv = np.ascontiguousarray(wqkvT[:, 2 * D:]).astype(bf)
    projT = np.ascontiguousarray(proj_w.T).astype(bf)
    qkb = np.ascontiguousarray(
        qkv_b[:2 * D].reshape(2 * ND, P).T).astype(np.float32)   # [128, 12]
    vb = qkv_b[2 * D:]
    pb2 = (proj_b + proj_w @ vb).astype(np.float32)
    pb2 = np.ascontiguousarray(pb2.reshape(ND, P).T)             # [128, 6]

    ident = np.eye(P, dtype=bf)
    in_maps = []
    for b in range(B):
        xT = np.ascontiguousarray(x[b].T).astype(bf)             # [768, 1024]
        in_maps.append({
            "xT": xT, "wqk": wqk, "wv": wv, "projT": projT,
            "qkb": qkb, "pb2": pb2, "ident": ident,
        })
    return in_maps


_CACHE = {}


def kernel(x, qkv_w, qkv_b, proj_w, proj_b):
    x = np.asarray(x, dtype=np.float32)
    qkv_w = np.asarray(qkv_w, dtype=np.float32)
    qkv_b = np.asarray(qkv_b, dtype=np.float32)
    proj_w = np.asarray(proj_w, dtype=np.float32)
    proj_b = np.asarray(proj_b, dtype=np.float32)

    if "nc" not in _CACHE:
        _CACHE["nc"] = build()
    nc = _CACHE["nc"]

    in_maps = prep_inputs(x, qkv_w, qkv_b, proj_w, proj_b)
    res = bass_utils.run_bass_kernel_spmd(nc, in_maps, core_ids=list(range(8)))
    out = np.empty((B, T, D), np.float32)
    for b in range(B):
        out[b] = res.results[b]["yT"].T.astype(np.float32)
    return out


if __name__ == "__main__":
    rng = np.random.default_rng(0)
    ins = {
        "x": rng.standard_normal((B, T, D), dtype=np.float32),
        "qkv_w": rng.standard_normal((3 * D, D), dtype=np.float32) * D ** -0.5,
        "qkv_b": rng.standard_normal(3 * D).astype(np.float32) * 0.02,
        "proj_w": rng.standard_normal((D, D), dtype=np.float32) * D ** -0.5,
        "proj_b": rng.standard_normal(D).astype(np.float32) * 0.02,
    }
    out = kernel(**ins)
    print("ok", out.shape, np.abs(out).max())


# revision 4
# speedup vs baseline: 1.0265x; 1.0265x over previous
"""Multi-head attention (B=8, T=1024, D=768, H=12) on 8 TRN2 NeuronCores.

Sharding: data-parallel over batch - one batch element per core, no
collectives.

v3: flat software-pipelined loop over 48 global (pair, tk) steps, with
all PE work packed via PE-array tiling (measured ~2.1x for pairs, ~4.5x
for the 4-way denominators):
  scores  : both heads row-paired (contraction 64 -> row groups 0:63 /
            64:127), 4 matmuls into one 4-bank [128,2048] PSUM tile
  exp     : ONE FD=2048 ACTIVATE per step (amortizes ACT overhead)
  attnV   : both heads col-paired into one PSUM tile (out partitions
            0:64 / 64:128)
  denoms  : ones[128,32] lhsT, 4-way col-tiled into one bank; every
            partition written so one DVE reciprocal covers all four
  fillers : V-proj halves, QK-proj halves (bias on DVE), and output-proj
            partial-contraction groups scheduled into specific steps;
            scores are emitted first in each step so exp never waits
  normalize: recip -> 3 row-extract DMAs -> 4 gpsimd broadcasts -> 2 DVE
            multiplies; attnV lags scores by a variable 4..6 steps so the
            chain never stalls the next pair's attnV (oacc/den WAR)
  tail    : yacc (ko 0..3, bf16) folded into PSUM via an identity matmul,
            ko4 prefilled during the last normalize chain, bf16 writeout

PSUM (8 banks): sc [128,2048] (4) + oacc [128,1024] (2) +
den [128,512] (1) + stg [128,512] (1).
"""

import numpy as np
import ml_dtypes

import concourse.bass as bass
import concourse.mybir as mybir
import concourse.tile as tile
from concourse import bacc
from concourse import bass_utils

BF16 = mybir.dt.bfloat16
F32 = mybir.dt.float32

B, T, D = 8, 1024, 768
H, HD = 12, 64
P = 128
ND = D // P           # 6 d-tiles
NT = T // P           # 8 t-tiles
NPAIR = H // 2        # 6 head pairs
NSTEP = NPAIR * NT    # 48 global steps
SCALE = HD ** -0.5
LAG = 5               # attnV lags scores by LAG global steps


def build():
    nc = bacc.Bacc("TRN2", target_bir_lowering=False, debug=False, num_devices=8)

    xT_d = nc.dram_tensor("xT", [D, T], BF16, kind="ExternalInput").ap()
    wqk_d = nc.dram_tensor("wqk", [D, 2 * D], BF16, kind="ExternalInput").ap()
    wv_d = nc.dram_tensor("wv", [D, D], BF16, kind="ExternalInput").ap()
    projT_d = nc.dram_tensor("projT", [D, D], BF16, kind="ExternalInput").ap()
    qkb_d = nc.dram_tensor("qkb", [P, 2 * ND], F32, kind="ExternalInput").ap()
    pb2_d = nc.dram_tensor("pb2", [P, ND], F32, kind="ExternalInput").ap()
    ident_d = nc.dram_tensor("ident", [P, P], BF16, kind="ExternalInput").ap()
    yT_d = nc.dram_tensor("yT", [D, T], BF16, kind="ExternalOutput").ap()

    with tile.TileContext(nc) as tc:
        with tc.tile_pool(name="const", bufs=1) as const, \
             tc.tile_pool(name="work", bufs=4) as work, \
             tc.tile_pool(name="psc", bufs=2, space="PSUM") as psc, \
             tc.tile_pool(name="pso", bufs=1, space="PSUM") as pso:

            # ---- resident SBUF tensors ----
            xT_sb = const.tile([P, ND, T], BF16)
            wv_sb = const.tile([P, ND, D], BF16)
            wqk_sb = const.tile([P, ND, 2 * D], BF16)
            projT_sb = const.tile([P, ND, D], BF16)
            qkb_sb = const.tile([P, 2 * ND], F32)
            pb2_sb = const.tile([P, ND], F32)
            QKT_sb = const.tile([P, 2 * ND, T], BF16)
            V_sb = const.tile([P, NT, D], BF16)
            aoT_sb = const.tile([P, ND, T], BF16)
            yacc_sb = const.tile([P, 2 * ND, 512], BF16)
            ones_sb = const.tile([P, 32], BF16)
            ident_sb = const.tile([P, P], BF16)

            nc.vector.memset(ones_sb[:], 1.0)
            nc.sync.dma_start(ident_sb[:], ident_d)

            # DMA priority: xT then the first Q/K weight tiles (unblocks the
            # first scores), then wv, then pair-1's Q/K tiles
            xT_r = xT_d.rearrange("(ko p) t -> p ko t", p=P)
            wv_r = wv_d.rearrange("(ko p) j -> p ko j", p=P)
            wqk_r = wqk_d.rearrange("(ko p) j -> p ko j", p=P)
            for c in range(3):
                nc.sync.dma_start(xT_sb[:, 2 * c:2 * c + 2, :],
                                  xT_r[:, 2 * c:2 * c + 2, :])
            for jt in (0, ND):
                nc.sync.dma_start(
                    wqk_sb[:, :, jt * P:(jt + 1) * P], wqk_r[:, :, jt * P:(jt + 1) * P])
            nc.sync.dma_start(qkb_sb[:], qkb_d)
            for c in range(3):
                nc.sync.dma_start(wv_sb[:, 2 * c:2 * c + 2, :],
                                  wv_r[:, 2 * c:2 * c + 2, :])
            for jt in (1, ND + 1):
                nc.sync.dma_start(
                    wqk_sb[:, :, jt * P:(jt + 1) * P], wqk_r[:, :, jt * P:(jt + 1) * P])

            # warm the exp table set immediately (no DMA dependency)
            warm = work.tile([1, 12], F32, tag="warm", bufs=1)
            nc.scalar.activation(warm[:], ones_sb[0:1, 0:12], mybir.ActivationFunctionType.Exp)


            def emit_v_half(t, jc):
                j0, jn = ((0, 512), (512, 256))[jc]
                ps = pso.tile([P, 512], F32, tag="stg", name=f"psvh_{t}_{jc}")
                for d in range(ND):
                    nc.tensor.matmul(
                        ps[:, :jn],
                        xT_sb[:, d, t * P:(t + 1) * P],
                        wv_sb[:, d, j0:j0 + jn],
                        start=(d == 0), stop=(d == ND - 1),
                    )
                nc.vector.tensor_copy(out=V_sb[:, t, j0:j0 + jn], in_=ps[:, :jn])

            def emit_qk_full(jt):
                ps = psc.tile([P, T], F32, tag="sc", name=f"psqk_{jt}", bufs=1)
                mm = None
                for d in range(ND):
                    for tq in range(2):
                        mm = nc.tensor.matmul(
                            ps[:, tq * 512:(tq + 1) * 512],
                            wqk_sb[:, d, jt * P:(jt + 1) * P],
                            xT_sb[:, d, tq * 512:(tq + 1) * 512],
                            start=(d == 0), stop=(d == ND - 1),
                        )
                nc.vector.tensor_scalar_add(
                    QKT_sb[:, jt, :], ps[:], qkb_sb[:, jt:jt + 1])
                return mm

            def emit_qk_half(jt, tq):
                ps = pso.tile([P, 512], F32, tag="stg", name=f"psqkh_{jt}_{tq}")
                for d in range(ND):
                    nc.tensor.matmul(
                        ps[:],
                        wqk_sb[:, d, jt * P:(jt + 1) * P],
                        xT_sb[:, d, tq * 512:(tq + 1) * 512],
                        start=(d == 0), stop=(d == ND - 1),
                    )
                nc.vector.tensor_scalar_add(
                    QKT_sb[:, jt, tq * 512:(tq + 1) * 512], ps[:],
                    qkb_sb[:, jt:jt + 1])

            def emit_proj(dt, tq, kos, first, last):
                # partial output-projection: accumulate contraction tiles kos
                # into yacc (bias folded on the first pass, in-place adds after)
                ps = pso.tile([P, 512], F32, tag="stg", name=f"pj_{dt}_{tq}_{kos[0]}")
                for n, ko in enumerate(kos):
                    nc.tensor.matmul(
                        ps[:],
                        projT_sb[:, ko, dt * P:(dt + 1) * P],
                        aoT_sb[:, ko, tq * 512:(tq + 1) * 512],
                        start=(n == 0), stop=(n == len(kos) - 1),
                    )
                ya = yacc_sb[:, 2 * dt + tq, :]
                if first:
                    nc.vector.tensor_scalar_add(ya, ps[:], pb2_sb[:, dt:dt + 1])
                else:
                    nc.vector.tensor_tensor(ya, ps[:], ya, mybir.AluOpType.add)

            # ---- upfront: pair-0 Q/K tiles (rides the input DMA) ----
            anchor = emit_qk_full(0)
            emit_qk_full(ND)

            # bulk weight DMAs gated behind the first QK tile (software-DGE)
            from concourse.tile_rust import add_dep_helper
            bulk = [
                nc.gpsimd.dma_start(
                    wqk_sb[:, :, 2 * P:ND * P], wqk_r[:, :, 2 * P:ND * P]),
                nc.gpsimd.dma_start(
                    wqk_sb[:, :, (ND + 2) * P:], wqk_r[:, :, (ND + 2) * P:]),
                nc.gpsimd.dma_start(
                    projT_sb[:], projT_d.rearrange("(ko p) j -> p ko j", p=P)),
                nc.gpsimd.dma_start(pb2_sb[:], pb2_d),
            ]
            for b in bulk:
                add_dep_helper(b.ins, anchor.ins, sync=True,
                               reason="bulk weight DMA after first QK tile")

            # ---- filler schedule: gk -> list of (fn, args) ----
            fillers = {}

            def sched(gk, fn, *args):
                fillers.setdefault(gk, []).append((fn, args))

            for t in range(4):                       # V t=0..3 early
                sched(t, emit_v_half, t, 0)
                sched(t, emit_v_half, t, 1)
            sched(4, emit_qk_half, 1, 0)             # pair-1 Q tile
            sched(4, emit_qk_half, 1, 1)
            sched(5, emit_qk_half, ND + 1, 0)        # pair-1 K tile
            sched(5, emit_qk_half, ND + 1, 1)
            for t in range(4, NT):                   # V t=4..7
                sched(2 + t, emit_v_half, t, 0)
                sched(2 + t, emit_v_half, t, 1)
            # normalize lands at gk 12,20,28,36,44 - keep those steps
            # filler-free so the stg ring never waits behind the boundary
            # DVE burst
            qk_slots = [10, 11, 13, 14, 15, 16, 17, 18, 19, 21, 22, 23,
                        24, 25, 26, 27]
            qk_jobs = [(jt, tq) for jt in (2, ND + 2, 3, ND + 3, 4, ND + 4,
                                           5, ND + 5) for tq in range(2)]
            for slot, (jt, tq) in zip(qk_slots, qk_jobs):
                sched(slot, emit_qk_half, jt, tq)
            # output projection partials: ko 0,1 after aoT(1) (~gk21);
            # ko 2,3 after aoT(3) (~gk37); ko 4,5 + writeout in the tail
            pj = [(dt, tq) for dt in range(ND) for tq in range(2)]
            p1_slots = [29, 29, 30, 30, 31, 31, 32, 32, 33, 34, 35, 37]
            p2_slots = [39, 39, 40, 40, 41, 41, 42, 42, 43, 43, 45, 45]
            for n, (dt, tq) in enumerate(pj):
                sched(p1_slots[n], emit_proj, dt, tq, (0, 1), True, False)
                sched(p2_slots[n], emit_proj, dt, tq, (2, 3), False, False)

            # ---- attention pipeline over 48 global steps ----
            pair_state = {}

            def emit_scores(gk):
                i, tk = divmod(gk, NT)
                if tk == 0:
                    oacc = pso.tile([P, T], F32, tag="oacc", name=f"oacc_{i}")
                    den = pso.tile([P, 512], F32, tag="den", name=f"den_{i}")
                    pair_state[i] = (oacc, den, {})
                at_tiles = pair_state[i][2]
                # one 4-bank score tile holds both heads -> a single FD=2048
                # exp per step (amortizes the ACT per-instruction overhead)
                sc = psc.tile([P, 2 * T], F32, tag="sc", name=f"sc_{i}_{tk}",
                              bufs=1)
                for tq in range(2):
                    for hh in range(2):
                        p0 = 64 * hh
                        nc.tensor.matmul(
                            sc[:, hh * T + tq * 512:hh * T + (tq + 1) * 512],
                            QKT_sb[p0:p0 + 64, ND + i, tk * P:(tk + 1) * P],
                            QKT_sb[p0:p0 + 64, i, tq * 512:(tq + 1) * 512],
                        )
                at = work.tile([P, 2 * T], BF16, tag="at",
                               name=f"at_{i}_{tk}", bufs=8)
                nc.scalar.activation(
                    at[:], sc[:], mybir.ActivationFunctionType.Exp,
                    scale=SCALE)
                at_tiles[tk] = at

            def emit_attnv(gk):
                i, tk = divmod(gk, NT)
                oacc, den, at_tiles = pair_state[i]
                at = at_tiles.pop(tk)
                for tq in range(2):
                    for hh in range(2):
                        h = 2 * i + hh
                        nc.tensor.matmul(
                            oacc[64 * hh:64 * hh + 64, tq * 512:(tq + 1) * 512],
                            V_sb[:, tk, h * HD:(h + 1) * HD],
                            at[:, hh * T + tq * 512:hh * T + (tq + 1) * 512],
                            start=(tk == 0), stop=(tk == NT - 1),
                        )
                for tq in range(2):
                    for hh in range(2):
                        g = hh + 2 * tq
                        nc.tensor.matmul(
                            den[32 * g:32 * g + 32, 0:512],
                            ones_sb[:, 0:32],
                            at[:, hh * T + tq * 512:hh * T + (tq + 1) * 512],
                            start=(tk == 0), stop=(tk == NT - 1),
                            tile_position=(0, 32 * g),
                            skip_group_check=True,
                        )

            def emit_normalize_a(i):
                # recip + row extraction + broadcasts (DVE cost: recip only)
                oacc, den, _ = pair_state[i]
                rsb = work.tile([P, 512], F32, tag="rsb", name=f"rsb_{i}", bufs=2)
                nc.vector.reciprocal_approx_fast(rsb[:], den[:])
                rext = work.tile([P, 3, 512], F32, tag="rext",
                                 name=f"rext_{i}", bufs=2)
                nc.sync.dma_start(rext[0:1, 0, :], rsb[32:33, :])
                nc.sync.dma_start(rext[0:1, 1, :], rsb[64:65, :])
                nc.sync.dma_start(rext[0:1, 2, :], rsb[96:97, :])
                rbc0 = work.tile([64, T], F32, tag="rbc0", name=f"rbc0_{i}", bufs=2)
                rbc1 = work.tile([64, T], F32, tag="rbc1", name=f"rbc1_{i}", bufs=2)
                nc.gpsimd.partition_broadcast(rbc0[:, 0:512], rsb[0:1, :])
                nc.gpsimd.partition_broadcast(rbc0[:, 512:1024], rext[0:1, 1, :])
                nc.gpsimd.partition_broadcast(rbc1[:, 0:512], rext[0:1, 0, :])
                nc.gpsimd.partition_broadcast(rbc1[:, 512:1024], rext[0:1, 2, :])
                pair_state[i] = (oacc, den, (rbc0, rbc1))

            def emit_normalize_b(i):
                # emitted ~2 steps after part a so independent filler DVE ops
                # run ahead of the broadcast-gated multiplies in the DVE FIFO
                oacc, den, (rbc0, rbc1) = pair_state[i]
                nc.vector.tensor_tensor(
                    aoT_sb[0:64, i, :], oacc[0:64, :], rbc0[:],
                    mybir.AluOpType.mult)
                nc.vector.tensor_tensor(
                    aoT_sb[64:128, i, :], oacc[64:128, :], rbc1[:],
                    mybir.AluOpType.mult)

            # variable attnV lag: finish a pair's attnV early (lag 4) and
            # start the next pair's late (lag 6) so the normalize chain gets
            # a ~3-step window before the next pair needs the oacc/den banks
            def lag_of(j):
                tk = j % NT
                return 4 if tk >= NT - 3 else (6 if tk <= 2 else 5)

            attnv_at = {}
            for j in range(NSTEP):
                attnv_at.setdefault(j + lag_of(j), []).append(j)

            for gk in range(NSTEP + 7):
                if gk < NSTEP:
                    emit_scores(gk)
                for fn, args in fillers.get(gk, []):
                    fn(*args)
                for j in attnv_at.get(gk, []):
                    emit_attnv(j)
                    if j % NT == NT - 1:
                        emit_normalize_a(j // NT)
                for j in attnv_at.get(gk - 2, []):
                    if j % NT == NT - 1:
                        emit_normalize_b(j // NT)

            # ---- tail: ko 4+5 as one PSUM group per output half, single
            # DVE add each. Rotate over 3 PSUM regions (stg + 2 freed sc
            # slots) with the ko4 matmuls software-pipelined 3 ahead so they
            # fill the last normalize-chain window (ko5 waits on aoT[5]).
            tail_ps = []

            def tail_bank(n):
                if n % 2 == 0:
                    return pso.tile([P, 512], F32, tag="stg", name=f"pt_{n}")
                return psc.tile([P, T], F32, tag="sc", name=f"pt_{n}",
                                bufs=1)[:, 0:512]

            def tail_ko4(n):
                dt, tq = pj[n]
                ps = tail_bank(n)
                tail_ps.append(ps)
                nc.tensor.matmul(
                    ps, projT_sb[:, 4, dt * P:(dt + 1) * P],
                    aoT_sb[:, 4, tq * 512:(tq + 1) * 512],
                    start=True, stop=False)
                # fold the ko0..3 partial in via an identity matmul so the
                # writeout needs only a (2x-mode) bf16 copy, no DVE add
                nc.tensor.matmul(
                    ps, ident_sb[:],
                    yacc_sb[:, 2 * dt + tq, :],
                    start=False, stop=False)

            def tail_ko5(n):
                dt, tq = pj[n]
                ps = tail_ps[n]
                nc.tensor.matmul(
                    ps, projT_sb[:, 5, dt * P:(dt + 1) * P],
                    aoT_sb[:, 5, tq * 512:(tq + 1) * 512],
                    start=False, stop=True)
                yt = work.tile([P, 512], BF16, tag="yt", name=f"ytt_{n}", bufs=4)
                nc.vector.tensor_copy(out=yt[:], in_=ps)
                nc.sync.dma_start(
                    yT_d[dt * P:(dt + 1) * P, tq * 512:(tq + 1) * 512], yt[:])

            tail_ko4(0)
            tail_ko4(1)
            tail_ko4(2)
            for n in range(12):
                tail_ko5(n)
                if n + 3 < 12:
                    tail_ko4(n + 3)

    nc.compile()
    return nc


def prep_inputs(x, qkv_w, qkv_b, proj_w, proj_b):
    """Host-side layout prep. Returns per-core input maps."""
    bf = ml_dtypes.bfloat16
    wqkvT = np.ascontiguousarray(qkv_w.T)          # [768, 2304] f32
    wqk = wqkvT[:, :2 * D].astype(bf)
    wv = np.ascontiguousarray(wqkvT[:, 2 * D:]).astype(bf)
    projT = np.ascontiguousarray(proj_w.T).astype(bf)
    qkb = np.ascontiguousarray(
        qkv_b[:2 * D].reshape(2 * ND, P).T).astype(np.float32)   # [128, 12]
    vb = qkv_b[2 * D:]
    pb2 = (proj_b + proj_w @ vb).astype(np.float32)
    pb2 = np.ascontiguousarray(pb2.reshape(ND, P).T)             # [128, 6]

    ident = np.eye(P, dtype=bf)
    in_maps = []
    for b in range(B):
        xT = np.ascontiguousarray(x[b].T).astype(bf)             # [768, 1024]
        in_maps.append({
            "xT": xT, "wqk": wqk, "wv": wv, "projT": projT,
            "qkb": qkb, "pb2": pb2, "ident": ident,
        })
    return in_maps


_CACHE = {}


def kernel(x, qkv_w, qkv_b, proj_w, proj_b):
    x = np.asarray(x, dtype=np.float32)
    qkv_w = np.asarray(qkv_w, dtype=np.float32)
    qkv_b = np.asarray(qkv_b, dtype=np.float32)
    proj_w = np.asarray(proj_w, dtype=np.float32)
    proj_b = np.asarray(proj_b, dtype=np.float32)

    if "nc" not in _CACHE:
        _CACHE["nc"] = build()
    nc = _CACHE["nc"]

    in_maps = prep_inputs(x, qkv_w, qkv_b, proj_w, proj_b)
    res = bass_utils.run_bass_kernel_spmd(nc, in_maps, core_ids=list(range(8)))
    out = np.empty((B, T, D), np.float32)
    for b in range(B):
        out[b] = res.results[b]["yT"].T.astype(np.float32)
    return out


if __name__ == "__main__":
    rng = np.random.default_rng(0)
    ins = {
        "x": rng.standard_normal((B, T, D), dtype=np.float32),
        "qkv_w": rng.standard_normal((3 * D, D), dtype=np.float32) * D ** -0.5,
        "qkv_b": rng.standard_normal(3 * D).astype(np.float32) * 0.02,
        "proj_w": rng.standard_normal((D, D), dtype=np.float32) * D ** -0.5,
        "proj_b": rng.standard_normal(D).astype(np.float32) * 0.02,
    }
    out = kernel(**ins)
    print("ok", out.shape, np.abs(out).max())
